# revision 23
# baseline (speedup 1.0000x reference)
"""Trainium2 Bass kernel for multi-head attention (b=2, n=2048, dim=1024,
heads=16, dim_head=64) sharded over 8 NeuronCores.

Sharding: core c handles batch c//4 and head group c%4 (4 heads).  Each core
computes its heads' full attention plus their slice of the output projection
(rows of w_out), producing a partial [n, dim] output; the host sums the four
partials per batch.  No collectives.

All matmuls in bf16 (rel err ~6e-3 vs the 2e-2 budget; fp8 was tested and
fails at 5e-2).  Layout per core:
  xT [1024, 2048] bf16           x^T, tokens in the free dim
  QT/KT pair tiles [128, 2048]   two heads stacked on the partition dim
  S^T [128j, 512i] pairs         the two K=64 score matmuls use row groups
                                 0:64 / 64:128 and distinct PSUM banks ->
                                 measured ~2x concurrent on HW
  P^T = exp(S^T) -> bf16         one ACT instr per [128, 1024] spair; ACT is
                                 the bottleneck engine (~134us of exp)
  V [tok, 64|ones64] per head    projected directly token-major (no PE
                                 transposes); 64 ones columns make AV rows
                                 64:128 the softmax denominator, so
                                 normalization is one reciprocal + one mul
                                 of [64, 512] per head
  y[i, 1024] += nt.T @ w_out     K=128 per pair, accumulated over pairs

Scheduling: projection / V / output-projection work is demoted to gap-filler
priority (tc.high_priority with negative offset) so the Tile scheduler keeps
the ACT-bound attention chain hot and fills PE bubbles with projection
matmuls; attention runs all pair-0 stripes before pair-1 so the pair-1
projections have 4 stripes of PE gaps to complete in.  PSUM budget: score
pairs 2x[128,1024] (4 banks) + AV accumulators oA/oB (2) + projection pool
2x[128,512] (2) = 8 banks.
"""

import os
import sys

import numpy as np

for _p in ("/opt/trn_rl_repo",):
    if _p not in sys.path and os.path.isdir(_p):
        sys.path.insert(0, _p)

import concourse.bass as bass  # noqa: E402
import concourse.mybir as mybir  # noqa: E402
import concourse.tile as tile  # noqa: E402
from concourse import bacc  # noqa: E402
from concourse import bass_utils  # noqa: E402

F32 = mybir.dt.float32
BF16 = mybir.dt.bfloat16
EXP = mybir.ActivationFunctionType.Exp
NPBF = mybir.dt.np(BF16)

B, N, DIM = 2, 2048, 1024
HEADS, DH = 16, 64
SCALE = DH ** -0.5
NCORES = 8
HPC = HEADS // (NCORES // B)  # heads per core = 4
NPAIRS = HPC // 2             # head pairs per core = 2

KC = DIM // 128               # contraction chunks for projections = 8
JC = N // 128                 # key/token blocks = 16
IQ = 4                        # query stripes
IQW = N // IQ                 # stripe width = 512

DEMOTE = -1000000             # priority offset for gap-filler work


def build_kernel(nc, tc, trips=1, variant="full"):
    xT = nc.dram_tensor("xT", [DIM, N], BF16, kind="ExternalInput").ap()
    mk = nc.dram_tensor("mk", [128, N], BF16, kind="ExternalInput").ap()
    wq = nc.dram_tensor("wq", [DIM, HPC * DH], BF16, kind="ExternalInput").ap()
    wk = nc.dram_tensor("wk", [DIM, HPC * DH], BF16, kind="ExternalInput").ap()
    wv = nc.dram_tensor("wv", [DIM, HPC * DH], BF16, kind="ExternalInput").ap()
    wo = nc.dram_tensor("wo", [HPC * DH, DIM], BF16, kind="ExternalInput").ap()
    y = nc.dram_tensor("y", [N, DIM], BF16, kind="ExternalOutput").ap()

    with (
        tc.tile_pool(name="pers", bufs=1) as pers,
        tc.tile_pool(name="ptp", bufs=3) as ptp,
        tc.tile_pool(name="ypool", bufs=2) as ypool,
        tc.tile_pool(name="pss", bufs=2, space="PSUM") as pss,
        tc.tile_pool(name="pj", bufs=2, space="PSUM") as pj,
        tc.tile_pool(name="pso", bufs=1, space="PSUM") as pso,
    ):
        xts = pers.tile([128, KC * N], BF16, tag="xts")
        qt = [pers.tile([128, N], BF16, tag=f"qt{p}", name=f"qt{p}")
              for p in range(NPAIRS)]
        kt = [pers.tile([128, N], BF16, tag=f"kt{p}", name=f"kt{p}")
              for p in range(NPAIRS)]
        v1 = pers.tile([128, HPC * JC * 128], BF16, tag="v1")
        wqs = pers.tile([128, KC * HPC * DH], BF16, tag="wqs")
        wks = pers.tile([128, KC * HPC * DH], BF16, tag="wks")
        wvs = pers.tile([128, KC * HPC * DH], BF16, tag="wvs")
        wos = pers.tile([128, NPAIRS * DIM], BF16, tag="wos")
        mks = pers.tile([128, N], BF16, tag="mks")

        v14 = v1.rearrange("p (h j c) -> p h j c", h=HPC, c=128)
        # ones columns 64:128 of every V block = softmax denominator source
        for h in range(HPC):
            nc.gpsimd.memset(v14[:, h, :, 64:128], 1.0)
        if variant == "attnonly":
            for t in (qt[0], qt[1], kt[0], kt[1]):
                nc.gpsimd.memset(t[:, :], 0.125)
            nc.gpsimd.memset(v1[:, :], 0.25)
        # prefetch the exp table before the timed body
        warm = pers.tile([1, 16], F32, tag="warm")
        nc.vector.memset(warm[:, :], 0.0)
        nc.scalar.activation(warm[:, :], warm[:, :], EXP)

        def proj(dst, w_t, p, masked):
            for nch in range(4):
                ps = pj.tile([128, 512], F32, tag="ps", name="ps")
                for kc in range(KC):
                    nc.tensor.matmul(
                        ps[:, :],
                        w_t[:, kc * 256 + p * 128: kc * 256 + p * 128 + 128],
                        xts[:, kc * N + nch * 512: kc * N + nch * 512 + 512],
                        start=(kc == 0), stop=(kc == KC - 1))
                dsl = dst[:, nch * 512:(nch + 1) * 512]
                if masked:
                    nc.vector.tensor_mul(dsl, ps[:, :],
                                         mks[:, nch * 512:(nch + 1) * 512])
                else:
                    nc.vector.tensor_copy(dsl, ps[:, :])

        def vblock(tb):
            # V for all 4 heads of token block tb, directly token-major
            psv = pj.tile([128, 512], F32, tag="ps", name="psv")
            for kc in range(KC):
                nc.tensor.matmul(
                    psv[:, 0:256],
                    xts[:, kc * N + tb * 128: kc * N + tb * 128 + 128],
                    wvs[:, kc * 256:(kc + 1) * 256],
                    start=(kc == 0), stop=(kc == KC - 1))
            pv = psv[:, 0:256].rearrange("p (h c) -> p h c", c=64)
            nc.vector.tensor_copy(v14[:, :, tb, 0:64], pv)

        def attnpair(iq, p):
            isl = slice(iq * IQW, (iq + 1) * IQW)
            oA = pso.tile([128, IQW], F32, tag="oA", name="oA")
            oB = pso.tile([128, IQW], F32, tag="oB", name="oB")
            for jc in range(JC):
                jsl = slice(jc * 128, (jc + 1) * 128)
                sp = pss.tile([128, 1024], F32, tag="s", name="sp")
                nc.tensor.matmul(sp[:, 0:512], kt[p][0:64, jsl],
                                 qt[p][0:64, isl], start=True, stop=True)
                nc.tensor.matmul(sp[:, 512:1024], kt[p][64:128, jsl],
                                 qt[p][64:128, isl], start=True, stop=True)
                pt = ptp.tile([128, 1024], BF16, tag="pt", name="pt", bufs=6)
                if variant == "noact":
                    nc.vector.tensor_copy(pt[:, 0:8], sp[:, 0:8])
                else:
                    nc.scalar.activation(pt[:, :], sp[:, :], EXP)
                nc.tensor.matmul(oA[:, :], v14[:, 2 * p, jc, :], pt[:, 0:512],
                                 start=(jc == 0), stop=(jc == JC - 1))
                nc.tensor.matmul(oB[:, :], v14[:, 2 * p + 1, jc, :],
                                 pt[:, 512:1024],
                                 start=(jc == 0), stop=(jc == JC - 1))
            # normalized out^T overwrites the dead q stripe
            d = ptp.tile([128, IQW], F32, tag="d", name="d", bufs=2)
            nc.vector.reciprocal(out=d[0:64, :], in_=oA[64:128, :])
            nc.vector.reciprocal(out=d[64:128, :], in_=oB[64:128, :])
            nc.vector.tensor_mul(qt[p][0:64, isl], oA[0:64, :], d[0:64, :])
            nc.vector.tensor_mul(qt[p][64:128, isl], oB[0:64, :], d[64:128, :])

        def yproj(iq):
            for ib in range(4):
                blk = iq * 4 + ib
                bsl = slice(blk * 128, (blk + 1) * 128)
                ysb = ypool.tile([128, DIM], BF16, tag="y", name="ysb")
                for nch2 in range(2):
                    yp = pj.tile([128, 512], F32, tag="ps", name="yp")
                    for p in range(NPAIRS):
                        nc.tensor.matmul(
                            yp[:, :], qt[p][:, bsl],
                            wos[:, p * DIM + nch2 * 512: p * DIM + nch2 * 512 + 512],
                            start=(p == 0), stop=(p == NPAIRS - 1))
                    nc.vector.tensor_copy(ysb[:, nch2 * 512:(nch2 + 1) * 512],
                                          yp[:, :])
                nc.sync.dma_start(out=y[bsl, :], in_=ysb[:, :])

        def dmas():
            qs = [nc.sync, nc.scalar, nc.gpsimd]
            for kc in range(KC):
                qs[kc % 3].dma_start(
                    out=xts[:, kc * N:(kc + 1) * N],
                    in_=xT[kc * 128:(kc + 1) * 128, :])
            nc.sync.dma_start(out=mks[:, :], in_=mk)
            for kc in range(KC):
                qs[kc % 3].dma_start(
                    out=wks[:, kc * 256:(kc + 1) * 256],
                    in_=wk[kc * 128:(kc + 1) * 128, :])
                qs[(kc + 1) % 3].dma_start(
                    out=wqs[:, kc * 256:(kc + 1) * 256],
                    in_=wq[kc * 128:(kc + 1) * 128, :])
                qs[(kc + 2) % 3].dma_start(
                    out=wvs[:, kc * 256:(kc + 1) * 256],
                    in_=wv[kc * 128:(kc + 1) * 128, :])
            for p in range(NPAIRS):
                nc.gpsimd.dma_start(out=wos[:, p * DIM:(p + 1) * DIM],
                                    in_=wo[p * 128:(p + 1) * 128, :])

        def body():
            attn_on = variant != "projonly"
            proj_on = variant != "attnonly"
            if variant != "nodma":
                dmas()
            if proj_on:
                with tc.high_priority(offset=DEMOTE):
                    proj(kt[0], wks, 0, True)
                    proj(qt[0], wqs, 0, False)
                    for tb in range(JC):
                        vblock(tb)
                    proj(kt[1], wks, 1, True)
                    proj(qt[1], wqs, 1, False)
            if attn_on:
                # all pair-0 stripes first: the demoted pair-1 projections
                # get 4 stripes of PE gaps before pair-1 attention needs them
                for iq in range(IQ):
                    attnpair(iq, 0)
                for iq in range(IQ):
                    attnpair(iq, 1)
                    if proj_on:
                        with tc.high_priority(offset=DEMOTE):
                            yproj(iq)
            elif proj_on:
                for iq in range(IQ):
                    yproj(iq)

        if variant == "nodma":
            dmas()
        if trips == 1:
            body()
        else:
            with tc.For_i(0, trips, 1, hint_engines=(
                    mybir.EngineType.PE, mybir.EngineType.Activation,
                    mybir.EngineType.DVE, mybir.EngineType.SP,
                    mybir.EngineType.Pool)):
                body()


_COMPILED = {}


def get_compiled(trips=1, variant="full"):
    key = (trips, variant)
    if key not in _COMPILED:
        nc = bacc.Bacc("TRN2", target_bir_lowering=False, debug=False,
                       num_devices=NCORES)
        with tile.TileContext(nc) as tc:
            build_kernel(nc, tc, trips=trips, variant=variant)
        nc.compile()
        _COMPILED[key] = nc
    return _COMPILED[key]


def make_in_maps(x, seq_mask, w_qkv, w_out):
    x = np.asarray(x, np.float32)
    seq_mask = np.asarray(seq_mask, np.float32)
    w_qkv = np.asarray(w_qkv, np.float32)
    w_out = np.asarray(w_out, np.float32)
    in_maps = []
    for c in range(NCORES):
        bc, g = divmod(c, NCORES // B)
        h0 = g * HPC * DH
        in_maps.append({
            "xT": np.ascontiguousarray(x[bc].T).astype(NPBF),
            "mk": np.ascontiguousarray(
                np.broadcast_to(seq_mask[bc], (128, N))).astype(NPBF),
            "wq": np.ascontiguousarray(
                w_qkv[:, h0:h0 + HPC * DH] * SCALE).astype(NPBF),
            "wk": np.ascontiguousarray(
                w_qkv[:, DIM + h0:DIM + h0 + HPC * DH]).astype(NPBF),
            "wv": np.ascontiguousarray(
                w_qkv[:, 2 * DIM + h0:2 * DIM + h0 + HPC * DH]).astype(NPBF),
            "wo": np.ascontiguousarray(w_out[h0:h0 + HPC * DH, :]).astype(NPBF),
        })
    return in_maps


LAST_RESULTS = None


def kernel(x, seq_mask, w_qkv, w_out, _trace=False, **trace_kwargs):
    global LAST_RESULTS
    nc = get_compiled()
    in_maps = make_in_maps(x, seq_mask, w_qkv, w_out)
    res = bass_utils.run_bass_kernel_spmd(
        nc, in_maps, core_ids=list(range(NCORES)), trace=_trace, **trace_kwargs)
    LAST_RESULTS = res
    out = np.zeros((B, N, DIM), np.float32)
    for c in range(NCORES):
        out[c // (NCORES // B)] += res.results[c]["y"].astype(np.float32)
    return out
